# revision 1
# baseline (speedup 1.0000x reference)
# kernel.py — AtomTransformerBlock on 8 TRN2 NeuronCores (SPMD, no collectives).
#
# Sharding: N_atom rows across 8 cores (256 rows each); x + weights replicated
# (each core recomputes LN(x), K, V for all 2048 rows). pair_emb sharded by
# first axis. All index-derived masks are precomputed on the host (pure index
# preprocessing); all tensor math happens on device.
#
# Per-core pipeline: pair-bias dma_gather (256B elements; reads 8MB instead of
# 64MB) -> LN1 -> PE transposes -> K^T/V^T/Q^T (fp32r matmuls) -> dense scores
# S[i,h,j] on PE -> bf16 [8h x 16jl] element layout -> DRAM -> dma_gather of
# 256B score elements at block_index positions -> one-hot-16 select -> softmax
# in [b-partition, (h,i)] space (PE ones-matmul partition reductions; host
# ln-multiplicity bias handles duplicate indices exactly) -> PE transpose ->
# local_scatter (per-partition indices, duplicates masked to -1) -> dense
# P[i,h,j] bf16 -> xbar DMA transpose -> dense AV on PE -> output projection,
# LN2, MLP in transposed (c-partition) space -> final transpose -> out.
#
# SBUF: tile tags are lifetime-shared aggressively (pool space is reserved per
# tag for the whole kernel). PSUM: tp(2x2KB) + mm(4x2KB) + av(2x2KB) = 16KB.
import math
import os
import sys

import numpy as np

sys.path.insert(0, "/opt/trn_rl_repo")

STAGE = int(os.environ.get("KSTAGE", "4"))

import ml_dtypes
from contextlib import ExitStack

import concourse.bass as bass
import concourse.mybir as mybir
import concourse.tile as tile
from concourse import bacc, library_config
from concourse.tile import add_dep_helper

N, C, H, DH, CP, B = 2048, 256, 8, 32, 32, 128
NCORES = 8
NSH = N // NCORES          # 256 rows per core
NBLK = NSH // 128          # 2 i-blocks per core
F32 = mybir.dt.float32
F32R = mybir.dt.float32r
BF16 = mybir.dt.bfloat16
I16 = mybir.dt.int16
AX = mybir.AxisListType
ALU = mybir.AluOpType
AF = mybir.ActivationFunctionType


def build_nc():
    from concourse.masks import make_identity

    nc = bacc.Bacc(None, target_bir_lowering=False, debug=True)

    x_d = nc.declare_dram_parameter("x", [N, C], F32, isOutput=False)
    xo_d = nc.declare_dram_parameter("xown", [NSH, C], F32, isOutput=False)
    pairs_d = nc.declare_dram_parameter("pairs", [NSH * 1024, 64], F32, isOutput=False)
    pgidx_d = nc.declare_dram_parameter("pgidx", [128, 8, 4, 64], I16, isOutput=False)
    pgpar_d = nc.declare_dram_parameter("pgpar", [128, 256], F32, isOutput=False)
    sgidx_d = nc.declare_dram_parameter("sgidx", [128, NBLK, 16, 64], I16, isOutput=False)
    oh16_d = nc.declare_dram_parameter("oh16", [128, NBLK, 128, 16], BF16, isOutput=False)
    lm_d = nc.declare_dram_parameter("lm", [128, NBLK, 128], F32, isOutput=False)
    scidx_d = nc.declare_dram_parameter("scidx", [128, NBLK, 2, 128], I16, isOutput=False)
    wqt_d = nc.declare_dram_parameter("wqt", [C, C], F32R, isOutput=False)
    wkt_d = nc.declare_dram_parameter("wkt", [C, C], F32R, isOutput=False)
    wvt_d = nc.declare_dram_parameter("wvt", [C, C], F32R, isOutput=False)
    wot_d = nc.declare_dram_parameter("wot", [C, C], F32R, isOutput=False)
    w1t_d = nc.declare_dram_parameter("w1t", [C, 4 * C], BF16, isOutput=False)
    w2t_d = nc.declare_dram_parameter("w2t", [4 * C, C], BF16, isOutput=False)
    b1p_d = nc.declare_dram_parameter("b1p", [128, 8], F32, isOutput=False)
    b2p_d = nc.declare_dram_parameter("b2p", [128, 2], F32, isOutput=False)
    wbc_d = nc.declare_dram_parameter("wbc", [128, 8], F32, isOutput=False)
    out_d = nc.declare_dram_parameter("out", [NSH, C], F32, isOutput=True)

    sdram = nc.dram_tensor("sdram", [NBLK, 128 * 128, 128], BF16)

    with tile.TileContext(nc) as tc, ExitStack() as ctx:
        pool = ctx.enter_context(tc.tile_pool(name="p", bufs=1))
        psmall = ctx.enter_context(tc.tile_pool(name="psm", bufs=1))
        pool3 = ctx.enter_context(tc.tile_pool(name="p3", bufs=2))
        psT = ctx.enter_context(tc.tile_pool(name="psT", bufs=2, space="PSUM"))
        psM = ctx.enter_context(tc.tile_pool(name="psM", bufs=4, space="PSUM"))
        psA = ctx.enter_context(tc.tile_pool(name="psA", bufs=2, space="PSUM"))

        ident = pool.tile([128, 128], F32)
        make_identity(nc, ident[:])

        def evac(dst_ap, src_ap, idx=0):
            if idx % 2 == 0:
                nc.vector.tensor_copy(dst_ap, src_ap)
            else:
                nc.scalar.activation(dst_ap, src_ap, AF.Copy)

        # ---------------- input loads ----------------
        X = pool.tile([128, 16, C], F32, tag="X")        # slot later reused by W2
        nc.sync.dma_start(out=X[:], in_=x_d[:].rearrange("(t p) c -> p t c", p=128))
        XOWN = pool.tile([128, NBLK, C], F32)
        nc.sync.dma_start(out=XOWN[:], in_=xo_d[:].rearrange("(t p) c -> p t c", p=128))
        WQ = pool.tile([128, 2, C], F32R)
        nc.sync.dma_start(out=WQ[:], in_=wqt_d[:].rearrange("(u p) c -> p u c", p=128))
        WK = pool.tile([128, 2, C], F32R)
        nc.sync.dma_start(out=WK[:], in_=wkt_d[:].rearrange("(u p) c -> p u c", p=128))
        WV = pool.tile([128, 2, C], F32R)
        nc.sync.dma_start(out=WV[:], in_=wvt_d[:].rearrange("(u p) c -> p u c", p=128))
        WO = pool.tile([128, 2, C], F32R)
        nc.sync.dma_start(out=WO[:], in_=wot_d[:].rearrange("(u p) c -> p u c", p=128))
        B1 = pool.tile([128, 8], F32)
        nc.sync.dma_start(out=B1[:], in_=b1p_d[:])
        B2 = pool.tile([128, 2], F32)
        nc.sync.dma_start(out=B2[:], in_=b2p_d[:])
        WBC = pool.tile([128, 8], F32)
        nc.sync.dma_start(out=WBC[:], in_=wbc_d[:])
        PGIDX = pool.tile([128, 8, 4, 64], I16, tag="PGIDX")
        nc.sync.dma_start(out=PGIDX[:], in_=pgidx_d[:])
        PGPAR = pool.tile([128, 256], F32)
        nc.sync.dma_start(out=PGPAR[:], in_=pgpar_d[:])
        SGIDX = pool.tile([128, NBLK, 16, 64], I16)
        nc.sync.dma_start(out=SGIDX[:], in_=sgidx_d[:])
        OH16 = pool.tile([128, NBLK, 128, 16], BF16)
        nc.sync.dma_start(out=OH16[:], in_=oh16_d[:])
        LM = pool.tile([128, NBLK, 128], F32)
        nc.sync.dma_start(out=LM[:], in_=lm_d[:])
        SCIDX = pool.tile([128, NBLK, 2, 128], I16)
        nc.sync.dma_start(out=SCIDX[:], in_=scidx_d[:])

        ones_col = pool.tile([128, 1], BF16)
        nc.vector.memset(ones_col[:], 1.0)
        ones_row = pool.tile([1, 128], BF16)
        nc.vector.memset(ones_row[:], 1.0)

        if STAGE >= 1:
            ll_mlp = nc.gpsimd.load_library(library_config.mlp)
        gathers = []

        # ---------------- pair-bias gather + reduce (two halves) -----------
        PMG = pool.tile([128, 256], F32)                 # [b, i]; /32 folded into wbc
        if STAGE < 1:
            nc.vector.memset(PMG[:], 0.001)
        for hf in range(2 if STAGE >= 1 else 0):
            GP = pool.tile([128, 128, 64], F32, tag="BIG", name=f"GP{hf}")
            for cc in range(4):
                c = hf * 4 + cc
                for q in range(4):
                    g = nc.gpsimd.dma_gather(
                        out_ap=GP[:, cc * 32 + q * 8:cc * 32 + (q + 1) * 8, :],
                        in_ap=pairs_d[c * 32768:(c + 1) * 32768, :],
                        idxs_ap=PGIDX[:, c, q, :],
                        num_idxs=1024,
                        num_idxs_reg=1024,
                        elem_size=64,
                    )
                    add_dep_helper(g.ins, ll_mlp.ins, reason="gather needs mlp lib")
                    gathers.append(g)
            R2 = psmall.tile([128, 128, 2], F32, tag="R2", name=f"R2{hf}")
            nc.vector.reduce_sum(R2[:], GP[:].rearrange("p i (t f) -> p i t f", t=2), axis=AX.X)
            sl = slice(hf * 128, (hf + 1) * 128)
            nc.vector.tensor_tensor(out=PMG[:, sl], in0=R2[:, :, 1], in1=R2[:, :, 0], op=ALU.subtract)
            nc.vector.tensor_tensor(out=PMG[:, sl], in0=PMG[:, sl], in1=PGPAR[:, sl], op=ALU.mult)
            nc.vector.tensor_tensor(out=PMG[:, sl], in0=PMG[:, sl], in1=R2[:, :, 0], op=ALU.add)

        # ---------------- LN1 ----------------
        def layernorm(dst, src, nt, nm):
            SQ = pool.tile([128, nt, C], F32, tag="BIG", name=f"SQ{nm}")
            nc.scalar.activation(SQ[:], src[:], AF.Square)
            RS1 = psmall.tile([128, nt], F32, tag="RS1", name=f"RS1{nm}")
            RS2 = psmall.tile([128, nt], F32, tag="RS2", name=f"RS2{nm}")
            nc.vector.reduce_sum(RS1[:], src[:], axis=AX.X)
            nc.vector.reduce_sum(RS2[:], SQ[:], axis=AX.X)
            MU = psmall.tile([128, nt], F32, tag="MU", name=f"MU{nm}")
            nc.vector.tensor_scalar_mul(out=MU[:], in0=RS1[:], scalar1=1.0 / C)
            VAR = psmall.tile([128, nt], F32, tag="VAR", name=f"VAR{nm}")
            nc.vector.tensor_scalar_mul(out=VAR[:], in0=RS2[:], scalar1=1.0 / C)
            MSQ = psmall.tile([128, nt], F32, tag="MSQ", name=f"MSQ{nm}")
            nc.vector.tensor_tensor(out=MSQ[:], in0=MU[:], in1=MU[:], op=ALU.mult)
            nc.vector.tensor_tensor(out=VAR[:], in0=VAR[:], in1=MSQ[:], op=ALU.subtract)
            nc.vector.tensor_scalar_add(out=VAR[:], in0=VAR[:], scalar1=1e-5)
            RSTD = psmall.tile([128, nt], F32, tag="RSTD", name=f"RSTD{nm}")
            nc.vector.reciprocal(RSTD[:], VAR[:])
            nc.scalar.activation(RSTD[:], RSTD[:], AF.Sqrt)
            nc.vector.tensor_tensor(
                out=dst[:], in0=src[:],
                in1=MU[:][:, :, None].to_broadcast([128, nt, C]),
                op=ALU.subtract)
            nc.vector.tensor_tensor(
                out=dst[:], in0=dst[:],
                in1=RSTD[:][:, :, None].to_broadcast([128, nt, C]),
                op=ALU.mult)

        XLN = pool.tile([128, 16, C], F32, tag="GSG", name="XLN")
        layernorm(XLN, X, 16, "a")
        XLNO = pool.tile([128, NBLK, C], F32)
        layernorm(XLNO, XOWN, NBLK, "b")

        # ---------------- transposes ----------------
        XT = pool.tile([128, 2, N], F32R, tag="XT")
        for t in range(16):
            for u in range(2):
                tp = psT.tile([128, 128], F32, tag="tp", name="tp")
                nc.tensor.transpose(out=tp[:], in_=XLN[:, t, u * 128:(u + 1) * 128], identity=ident[:])
                evac(XT[:, u, t * 128:(t + 1) * 128], tp[:], t + u)
        XQT = pool.tile([128, 2, NSH], F32R)
        XOT = pool.tile([128, 2, NSH], F32)
        for t in range(NBLK):
            for u in range(2):
                tp = psT.tile([128, 128], F32, tag="tp", name="tp")
                nc.tensor.transpose(out=tp[:], in_=XLNO[:, t, u * 128:(u + 1) * 128], identity=ident[:])
                evac(XQT[:, u, t * 128:(t + 1) * 128], tp[:], t + u)
                tp2 = psT.tile([128, 128], F32, tag="tp", name="tp2")
                nc.tensor.transpose(out=tp2[:], in_=XOWN[:, t, u * 128:(u + 1) * 128], identity=ident[:])
                evac(XOT[:, u, t * 128:(t + 1) * 128], tp2[:], t + u + 1)

        # ---------------- K^T, V^T, Q^T ----------------
        KT = pool.tile([128, 2, N], F32R, tag="KT")
        VTB = pool.tile([128, 2, N], BF16, tag="VTB")
        for ch in range(2):
            for jc in range(4):
                kp = psM.tile([128, 512], F32, tag="mm", name="kp")
                vp = psM.tile([128, 512], F32, tag="mm", name="vp")
                for u in range(2):
                    nc.tensor.matmul(
                        kp[:], WK[:, u, ch * 128:(ch + 1) * 128],
                        XT[:, u, jc * 512:(jc + 1) * 512],
                        start=(u == 0), stop=(u == 1))
                for u in range(2):
                    nc.tensor.matmul(
                        vp[:], WV[:, u, ch * 128:(ch + 1) * 128],
                        XT[:, u, jc * 512:(jc + 1) * 512],
                        start=(u == 0), stop=(u == 1))
                evac(KT[:, ch, jc * 512:(jc + 1) * 512], kp[:], jc)
                evac(VTB[:, ch, jc * 512:(jc + 1) * 512], vp[:], jc + 1)
        QT = pool.tile([128, 2, NSH], F32R)
        for ch in range(2):
            qp = psM.tile([128, 512], F32, tag="mm", name="qp")
            for u in range(2):
                nc.tensor.matmul(
                    qp[:, :NSH], WQ[:, u, ch * 128:(ch + 1) * 128],
                    XQT[:, u, :],
                    start=(u == 0), stop=(u == 1))
            evac(QT[:, ch, :], qp[:, :NSH], ch)

        VR = pool.tile([128, 16, C], BF16, tag="VR")
        for ch in range(2):
            nc.sync.dma_start(out=VR[:, :, ch * 128:(ch + 1) * 128], in_=VTB[:, ch, :], transpose=True)

        if STAGE >= 2:
            _dense_scores = True
        # ---------------- dense scores (both blocks) ----------------
        for blk in range(NBLK if STAGE >= 2 else 0):
            S_sb = pool.tile([128, 128, 8, 16], BF16, tag="BIG", name=f"S_sb{blk}")
            for h in range(H):
                qs = QT[(h % 4) * 32:(h % 4) * 32 + 32, h // 4, blk * 128:(blk + 1) * 128]
                for jc in range(4):
                    sp = psM.tile([128, 512], F32, tag="mm", name="sp")
                    nc.tensor.matmul(
                        sp[:], qs,
                        KT[(h % 4) * 32:(h % 4) * 32 + 32, h // 4, jc * 512:(jc + 1) * 512],
                        start=True, stop=True, tile_position=((h % 4) * 32, 0))
                    evac(S_sb[:, jc * 32:(jc + 1) * 32, h, :],
                         sp[:].rearrange("p (jb jl) -> p jb jl", jl=16), h + jc)
            nc.sync.dma_start(
                out=sdram[blk].rearrange("(i r) e -> i (r e)", i=128),
                in_=S_sb[:].rearrange("p jb h jl -> p (jb h jl)"))

        # ---------------- gather-back + select + softmax (both blocks) -----
        WTs = []
        for blk in range(NBLK if STAGE >= 2 else 0):
            G_SG = pool.tile([128, 128, 128], BF16, tag="GSG", name=f"G_SG{blk}")
            for q in range(16):
                g = nc.gpsimd.dma_gather(
                    out_ap=G_SG[:, q * 8:(q + 1) * 8, :],
                    in_ap=sdram[blk],
                    idxs_ap=SGIDX[:, blk, q, :],
                    num_idxs=1024,
                    num_idxs_reg=1024,
                    elem_size=128,
                )
                add_dep_helper(g.ins, ll_mlp.ins, reason="gather needs mlp lib")
                gathers.append(g)

            # one-hot-16 select -> ST [b, h, i]; ST/E/RB live in KT's slot
            SMX = pool.tile([128, 3072], F32, tag="KT", name=f"SMX{blk}")
            ST = SMX[:, 0:1024].rearrange("p (h i) -> p h i", h=8)
            E = SMX[:, 1024:2048].rearrange("p (h i) -> p h i", h=8)
            RB = SMX[:, 2048:3072].rearrange("p (h i) -> p h i", h=8)
            for h in range(H):
                MS = pool3.tile([128, 128, 16], BF16, tag="MS", name="MS")
                nc.vector.tensor_tensor(
                    out=MS[:], in0=G_SG[:].rearrange("p i (h k) -> p i h k", k=16)[:, :, h, :],
                    in1=OH16[:, blk], op=ALU.mult)
                nc.vector.reduce_sum(ST[:, h, :], MS[:], axis=AX.X)

            # softmax in [b, (h, i)] space
            nc.vector.tensor_tensor(
                out=ST, in0=ST,
                in1=LM[:, blk, :][:, None, :].to_broadcast([128, 8, 128]),
                op=ALU.add)
            T1 = pool.tile([128, 8, 128], F32, tag="XT2", name=f"T1{blk}")
            nc.vector.tensor_tensor(
                out=T1[:],
                in0=PMG[:, blk * 128:(blk + 1) * 128][:, None, :].to_broadcast([128, 8, 128]),
                in1=WBC[:][:, :, None].to_broadcast([128, 8, 128]),
                op=ALU.mult)
            nc.vector.tensor_tensor(out=ST, in0=ST, in1=T1[:], op=ALU.add)
            nc.scalar.activation(E, ST, AF.Exp)
            EB = psmall.tile([128, 1024], BF16, tag="EB", name=f"EB{blk}")
            nc.vector.tensor_copy(EB[:], SMX[:, 1024:2048])
            Ef = EB[:]
            Dps = []
            for half in range(2):
                Dp = psM.tile([1, 512], F32, tag="mm", name=f"Dp{half}")
                nc.tensor.matmul(
                    Dp[:], ones_col[:],
                    Ef[:, half * 512:(half + 1) * 512],
                    start=True, stop=True)
                Dps.append(Dp)
            Rcp = pool.tile([1, 1024], F32, tag="PGIDX", name=f"Rcp{blk}")
            for half in range(2):
                nc.vector.reciprocal(Rcp[:, half * 512:(half + 1) * 512], Dps[half][:])
            RBf = SMX[:, 2048:3072]
            RcpB = psmall.tile([1, 1024], BF16, tag="RcpB", name=f"RcpB{blk}")
            nc.vector.tensor_copy(RcpB[:], Rcp[:])
            for half in range(2):
                RBp = psM.tile([128, 512], F32, tag="mm", name=f"RBp{half}")
                nc.tensor.matmul(
                    RBp[:], ones_row[:],
                    RcpB[:, half * 512:(half + 1) * 512],
                    start=True, stop=True)
                evac(RBf[:, half * 512:(half + 1) * 512], RBp[:], half)
            WSM = pool.tile([128, 8, 128], F32, tag="VTB", name=f"WSM{blk}")
            nc.vector.tensor_tensor(out=WSM[:], in0=E, in1=RB, op=ALU.mult)

            WT = pool.tile([128, 8, 128], BF16, tag=f"WT{blk}", name=f"WT{blk}")
            for h in range(H):
                tp = psT.tile([128, 128], F32, tag="tp", name="tp")
                nc.tensor.transpose(out=tp[:], in_=WSM[:, h, :], identity=ident[:])
                evac(WT[:, h, :], tp[:], h)
            WTs.append(WT)

        chain_depth = int(os.environ.get("KCHAIN", "2"))
        if chain_depth > 0:
            for n in range(chain_depth, len(gathers)):
                add_dep_helper(gathers[n].ins, gathers[n - chain_depth].ins, sync=True,
                               reason="swdge ring reclaim chain")

        # gpsimd switches libraries after ALL gathers
        if STAGE >= 3:
            ll_ls = nc.gpsimd.load_library(library_config.local_scatter)
            for g in gathers:
                add_dep_helper(ll_ls.ins, g.ins, reason="lib switch after gathers")

        # ---------------- scatter + P^T + AV per block ----------------
        ATT = pool.tile([128, 2, NSH], F32R, tag="ATT")
        if STAGE < 3:
            nc.vector.memset(ATT[:].bitcast(F32), 0.001)
        for blk in range(NBLK if STAGE >= 3 else 0):
            WT = WTs[blk]
            PD = pool.tile([128, 8, 2048], BF16, tag="BIG", name=f"PD{blk}")
            for h in range(H):
                for half in range(2):
                    ls = nc.gpsimd.local_scatter(
                        out_ap=PD[:, h, half * 1024:(half + 1) * 1024],
                        data_ap=WT[:, h, :],
                        idxs_ap=SCIDX[:, blk, half, :],
                        channels=128,
                        num_elems=1024,
                        num_idxs=128,
                    )
                    add_dep_helper(ls.ins, ll_ls.ins, reason="scatter needs ls lib")

            for hg in range(2):
                for k in range(4):
                    h = hg * 4 + k
                    PT = pool3.tile([128, 16, 128], BF16, tag="MS", name="PT")
                    nc.sync.dma_start(out=PT[:], in_=PD[:, h, :], transpose=True)
                    av = psA.tile([128, 128], F32, tag="av", name="av")
                    for jh in range(16):
                        nc.tensor.matmul(
                            av[:], VR[:, jh, hg * 128:(hg + 1) * 128],
                            PT[:, jh, :],
                            start=(jh == 0), stop=(jh == 15))
                    evac(ATT[k * 32:(k + 1) * 32, hg, blk * 128:(blk + 1) * 128],
                         av[k * 32:(k + 1) * 32, :], k)

        # ---------------- output projection + residual ----------------
        Y1T = pool.tile([128, 2, NSH], F32, tag="Y1T")
        for ch in range(2):
            op_ = psM.tile([128, 512], F32, tag="mm", name="op")
            for u in range(2):
                nc.tensor.matmul(
                    op_[:, :NSH], WO[:, u, ch * 128:(ch + 1) * 128],
                    ATT[:, u, :],
                    start=(u == 0), stop=(u == 1))
            nc.vector.tensor_tensor(out=Y1T[:, ch, :], in0=op_[:, :NSH], in1=XOT[:, ch, :], op=ALU.add)

        # ---------------- LN2 ----------------
        Y1 = pool.tile([128, NBLK, C], F32, tag="Y1")
        for t in range(NBLK):
            for ch in range(2):
                tp = psT.tile([128, 128], F32, tag="tp", name="tp")
                nc.tensor.transpose(out=tp[:], in_=Y1T[:, ch, t * 128:(t + 1) * 128], identity=ident[:])
                evac(Y1[:, t, ch * 128:(ch + 1) * 128], tp[:], t + ch)
        H2 = pool.tile([128, NBLK, C], F32, tag="H2")
        layernorm(H2, Y1, NBLK, "c")
        H2T = pool.tile([128, 2, NSH], BF16, tag="H2T")
        for t in range(NBLK):
            for ch in range(2):
                tp = psT.tile([128, 128], F32, tag="tp", name="tp")
                nc.tensor.transpose(out=tp[:], in_=H2[:, t, ch * 128:(ch + 1) * 128], identity=ident[:])
                evac(H2T[:, ch, t * 128:(t + 1) * 128], tp[:], t + ch)

        # ---------------- MLP (W1/W2 loaded late into freed slots) --------
        W1 = pool.tile([128, 2, 4 * C], BF16, tag="GSG", name="W1")
        nc.sync.dma_start(out=W1[:], in_=w1t_d[:].rearrange("(u p) c -> p u c", p=128))
        W2 = pool.tile([128, 8, C], BF16, tag="X", name="W2")
        nc.sync.dma_start(out=W2[:], in_=w2t_d[:].rearrange("(u p) c -> p u c", p=128))
        M1 = pool.tile([128, 8, NSH], BF16, tag="XT2", name="M1")
        for mc in range(8):
            mp = psM.tile([128, 512], F32, tag="mm", name="mp")
            for u in range(2):
                nc.tensor.matmul(
                    mp[:, :NSH], W1[:, u, mc * 128:(mc + 1) * 128],
                    H2T[:, u, :],
                    start=(u == 0), stop=(u == 1))
            TM = psmall.tile([128, NSH], BF16, tag="TM", name="TM")
            nc.vector.tensor_scalar_add(out=TM[:], in0=mp[:, :NSH], scalar1=B1[:, mc:mc + 1])
            nc.scalar.activation(M1[:, mc, :], mp[:, :NSH], AF.Sigmoid, bias=B1[:, mc:mc + 1])
            nc.vector.tensor_tensor(out=M1[:, mc, :], in0=M1[:, mc, :], in1=TM[:], op=ALU.mult)
        YT = pool.tile([128, 2, NSH], F32, tag="YT")
        for ch in range(2):
            yp = psM.tile([128, 512], F32, tag="mm", name="yp")
            for mc in range(8):
                nc.tensor.matmul(
                    yp[:, :NSH], W2[:, mc, ch * 128:(ch + 1) * 128],
                    M1[:, mc, :],
                    start=(mc == 0), stop=(mc == 7))
            nc.vector.tensor_scalar_add(out=YT[:, ch, :], in0=yp[:, :NSH], scalar1=B2[:, ch:ch + 1])
            nc.vector.tensor_tensor(out=YT[:, ch, :], in0=YT[:, ch, :], in1=Y1T[:, ch, :], op=ALU.add)

        # ---------------- final transpose + store ----------------
        OUT = pool.tile([128, NBLK, C], F32, tag="OUT")
        for t in range(NBLK):
            for ch in range(2):
                tp = psT.tile([128, 128], F32, tag="tp", name="tp")
                nc.tensor.transpose(out=tp[:], in_=YT[:, ch, t * 128:(t + 1) * 128], identity=ident[:])
                evac(OUT[:, t, ch * 128:(ch + 1) * 128], tp[:], t + ch)
        nc.sync.dma_start(out=out_d[:].rearrange("(t p) c -> p t c", p=128), in_=OUT[:])

    nc.compile()
    return nc


# ======================= host side =======================

def _wrap16(flat):
    """dma_gather index layout: idx k at [k%16, k//16], replicated x8 groups."""
    n = flat.shape[0]
    arr = flat.reshape(n // 16, 16).T.astype(np.int16)
    return np.tile(arr, (8, 1))


def _host_prep(core, x, pair_emb, block_index, Wq, Wk, Wv, Wb, Wout, W1, b1, W2, b2):
    i0 = core * NSH
    idx = np.asarray(block_index[i0:i0 + NSH]).astype(np.int64)   # [256, 128]

    pgidx = np.zeros((128, 8, 4, 64), np.int16)
    for c in range(8):
        sub = idx[c * 32:(c + 1) * 32]                            # [32, 128]
        flat = (np.arange(32)[:, None] * 1024 + sub // 2).reshape(-1)
        for q in range(4):
            pgidx[:, c, q, :] = _wrap16(flat[q * 1024:(q + 1) * 1024])
    pgpar = np.ascontiguousarray((idx % 2).astype(np.float32).T)  # [b, i]

    sgidx = np.zeros((128, NBLK, 16, 64), np.int16)
    oh16 = np.zeros((128, NBLK, 128, 16), ml_dtypes.bfloat16)
    lm = np.zeros((128, NBLK, 128), np.float32)
    scidx = np.zeros((128, NBLK, 2, 128), np.int16)
    for blk in range(NBLK):
        sub = idx[blk * 128:(blk + 1) * 128]                      # [128 i, 128 b]
        flat = (np.arange(128)[:, None] * 128 + sub // 16).reshape(-1)
        for q in range(16):
            sgidx[:, blk, q, :] = _wrap16(flat[q * 1024:(q + 1) * 1024])
        oh = np.zeros((128, 128, 16), np.float32)                 # [b, i, k]
        ii = np.repeat(np.arange(128), 128)
        bb = np.tile(np.arange(128), 128)
        oh[bb, ii, (sub % 16).reshape(-1)] = 1.0
        oh16[:, blk] = oh.astype(ml_dtypes.bfloat16)
        for i in range(128):
            row = sub[i]
            uniq, first_pos, counts = np.unique(row, return_index=True, return_counts=True)
            lmrow = np.full(128, -30.0, np.float32)
            scrow = np.full(128, -1, np.int64)
            lmrow[first_pos] = np.log(counts.astype(np.float64)).astype(np.float32)
            scrow[first_pos] = uniq
            lm[:, blk, i] = lmrow
            for half in range(2):
                sc = np.where((scrow >= 1024 * half) & (scrow < 1024 * (half + 1)),
                              scrow - 1024 * half, -1)
                scidx[i, blk, half, :] = sc.astype(np.int16)

    scale = 1.0 / math.sqrt(DH)
    fp = np.float32
    feeds = {
        "x": np.ascontiguousarray(x, fp),
        "xown": np.ascontiguousarray(np.asarray(x, fp)[i0:i0 + NSH]),
        "pairs": np.ascontiguousarray(
            np.asarray(pair_emb[i0:i0 + NSH], fp).reshape(NSH * 1024, 64)),
        "pgidx": pgidx, "pgpar": pgpar, "sgidx": sgidx,
        "oh16": np.asarray(oh16), "lm": lm, "scidx": scidx,
        "wqt": np.ascontiguousarray(np.asarray(Wq, fp).T * scale),
        "wkt": np.ascontiguousarray(np.asarray(Wk, fp).T),
        "wvt": np.ascontiguousarray(np.asarray(Wv, fp).T),
        "wot": np.ascontiguousarray(np.asarray(Wout, fp).T),
        "w1t": np.ascontiguousarray(np.asarray(W1, fp).T).astype(ml_dtypes.bfloat16),
        "w2t": np.ascontiguousarray(np.asarray(W2, fp).T).astype(ml_dtypes.bfloat16),
        "b1p": np.ascontiguousarray(np.asarray(b1, fp).reshape(8, 128).T),
        "b2p": np.ascontiguousarray(np.asarray(b2, fp).reshape(2, 128).T),
        "wbc": np.tile(np.asarray(Wb, fp).reshape(1, 8) / CP, (128, 1)),
    }
    return feeds


_NC = None


def kernel(**inputs):
    global _NC
    from concourse.bass_utils import run_bass_kernel_spmd
    if _NC is None:
        _NC = build_nc()
    in_maps = [_host_prep(core, **inputs) for core in range(NCORES)]
    r = run_bass_kernel_spmd(_NC, in_maps, core_ids=list(range(NCORES)))
    out = np.concatenate([np.asarray(r.results[i]["out"]).reshape(NSH, C)
                          for i in range(NCORES)], axis=0)
    return out.astype(np.float32)



# revision 38
# speedup vs baseline: 545.8477x; 545.8477x over previous
# kernel.py — AtomTransformerBlock on 8 TRN2 NeuronCores (SPMD, no collectives).
#
# Sharding: N_atom rows across 8 cores (256 rows each); x + weights replicated
# (each core recomputes LN(x), K, V for all 2048 rows). pair_emb sharded by
# first axis. All index-derived masks are precomputed on the host (pure index
# preprocessing); all tensor math happens on device.
#
# Per-core pipeline (v2 — in-SBUF reverse-scatter selection, no DRAM
# round-trip for scores):
#   pair-bias dma_gather (256B f32 elements; 8MB instead of 64MB) -> parity
#   select -> PMG [b, i] -> PE-transpose -> PMGT [i, b].
#   LN1 (bf16, per-column tensor_scalar apply) -> PE transposes (bf16) ->
#   K^T/V^T/Q^T bf16 matmuls -> dense scores S_sb [i, h, j] bf16 in SBUF ->
#   per (blk, h) REVERSE local_scatter (idx maps dense j -> block slot b,
#   -1 elsewhere; duplicates get slot only at first occurrence) -> STT
#   [i, h, b] -> softmax along the FREE b axis (host ln-multiplicity bias
#   handles duplicate indices exactly; dup slots scatter to 0 and are killed
#   by the -30 bias) -> W [i, h, b] feeds local_scatter directly (no
#   transposes) -> dense P [i, h, j] bf16 -> xbar DMA transpose -> dense AV
#   on PE -> per-block output projection, LN2, MLP in transposed (c-partition)
#   space -> final transpose -> out.
#
# Emission order is tuned for the in-order engine queues: LN1 before the
# pair reduces (DVE), PMGT transposes after the K/V/Q matmuls (PE), both
# blocks' reverse scatters back-to-back (Pool), per-block tail.
#
# SBUF: tile tags are lifetime-shared aggressively. PSUM: tp(2) + mm(4) +
# av(2) = 8 banks.
import math
import os
import sys

import numpy as np

sys.path.insert(0, "/opt/trn_rl_repo")

STAGE = int(os.environ.get("KSTAGE", "4"))

import ml_dtypes
from contextlib import ExitStack

import concourse.bass as bass
import concourse.mybir as mybir
import concourse.tile as tile
from concourse import bacc, library_config
from concourse.tile import add_dep_helper

N, C, H, DH, CP, B = 2048, 256, 8, 32, 32, 128
NCORES = 8
NSH = N // NCORES          # 256 rows per core
NBLK = NSH // 128          # 2 i-blocks per core
F32 = mybir.dt.float32
F32R = mybir.dt.float32r
BF16 = mybir.dt.bfloat16
I16 = mybir.dt.int16
AX = mybir.AxisListType
ALU = mybir.AluOpType
AF = mybir.ActivationFunctionType


def build_nc():
    from concourse.masks import make_identity

    nc = bacc.Bacc(None, target_bir_lowering=False, debug=True)

    x_d = nc.declare_dram_parameter("x", [N, C], BF16, isOutput=False)
    xo_d = nc.declare_dram_parameter("xown", [NSH, C], F32, isOutput=False)
    pairs_d = nc.declare_dram_parameter("pairs", [NSH * 1024, 64], F32, isOutput=False)
    pgidx_d = nc.declare_dram_parameter("pgidx", [128, 8, 4, 64], I16, isOutput=False)
    pgpar_d = nc.declare_dram_parameter("pgpar", [128, 256], F32, isOutput=False)
    rsc_d = nc.declare_dram_parameter("rsc", [128, NBLK, 2, 1024], I16, isOutput=False)
    lmt_d = nc.declare_dram_parameter("lmt", [128, NBLK, 128], BF16, isOutput=False)
    scidx_d = nc.declare_dram_parameter("scidx", [128, NBLK, 2, 128], I16, isOutput=False)
    wqt_d = nc.declare_dram_parameter("wqt", [C, C], BF16, isOutput=False)
    wkt_d = nc.declare_dram_parameter("wkt", [C, C], BF16, isOutput=False)
    wvt_d = nc.declare_dram_parameter("wvt", [C, C], BF16, isOutput=False)
    wot_d = nc.declare_dram_parameter("wot", [C, C], BF16, isOutput=False)
    w1t_d = nc.declare_dram_parameter("w1t", [C, 4 * C], BF16, isOutput=False)
    w2t_d = nc.declare_dram_parameter("w2t", [4 * C, C], BF16, isOutput=False)
    b1e_d = nc.declare_dram_parameter("b1e", [128, 1024], BF16, isOutput=False)
    b2p_d = nc.declare_dram_parameter("b2p", [128, 2], F32, isOutput=False)
    wbc_d = nc.declare_dram_parameter("wbc", [128, 8], F32, isOutput=False)
    out_d = nc.declare_dram_parameter("out", [NSH, C], F32, isOutput=True)

    with tile.TileContext(nc) as tc, ExitStack() as ctx:
        pool = ctx.enter_context(tc.tile_pool(name="p", bufs=1))
        psmall = ctx.enter_context(tc.tile_pool(name="psm", bufs=1))
        pool3 = ctx.enter_context(tc.tile_pool(name="p3", bufs=2))
        psT = ctx.enter_context(tc.tile_pool(name="psT", bufs=2, space="PSUM"))
        psM = ctx.enter_context(tc.tile_pool(name="psM", bufs=2, space="PSUM"))
        psA = ctx.enter_context(tc.tile_pool(name="psA", bufs=2, space="PSUM"))

        identf = pool.tile([128, 128], F32)
        make_identity(nc, identf[:])
        identb = pool.tile([128, 128], BF16)
        make_identity(nc, identb[:])

        def evac(dst_ap, src_ap, idx=0):
            if idx % 2 == 0:
                return nc.vector.tensor_copy(dst_ap, src_ap)
            else:
                return nc.scalar.activation(dst_ap, src_ap, AF.Copy)

        # ---------------- input loads (PGIDX first: pair gathers gate) ----
        PGIDX = pool.tile([128, 8, 4, 64], I16, tag="PGIDX")
        nc.sync.dma_start(out=PGIDX[:], in_=pgidx_d[:])
        X = pool.tile([128, 16, C], BF16, tag="X")       # slot later reused by W2
        nc.sync.dma_start(out=X[:], in_=x_d[:].rearrange("(t p) c -> p t c", p=128))
        XOWN = pool.tile([128, NBLK, C], F32)
        nc.sync.dma_start(out=XOWN[:], in_=xo_d[:].rearrange("(t p) c -> p t c", p=128))
        WQ = pool.tile([128, 2, C], BF16)
        nc.sync.dma_start(out=WQ[:], in_=wqt_d[:].rearrange("(u p) c -> p u c", p=128))
        WK = pool.tile([128, 2, C], BF16)
        nc.sync.dma_start(out=WK[:], in_=wkt_d[:].rearrange("(u p) c -> p u c", p=128))
        WV = pool.tile([128, 2, C], BF16)
        nc.sync.dma_start(out=WV[:], in_=wvt_d[:].rearrange("(u p) c -> p u c", p=128))
        WO = pool.tile([128, 2, C], BF16)
        nc.sync.dma_start(out=WO[:], in_=wot_d[:].rearrange("(u p) c -> p u c", p=128))
        B2 = pool.tile([128, 2], F32)
        nc.sync.dma_start(out=B2[:], in_=b2p_d[:])
        WBC = pool.tile([128, 8], F32)
        nc.sync.dma_start(out=WBC[:], in_=wbc_d[:])
        PGPAR = pool.tile([128, 256], F32)
        nc.sync.dma_start(out=PGPAR[:], in_=pgpar_d[:])
        RSCs = []
        for blk in range(NBLK):
            RSCb = pool.tile([128, 2, 1024], I16, tag="RSC", name=f"RSC{blk}")
            nc.sync.dma_start(out=RSCb[:], in_=rsc_d[:, blk])
            RSCs.append(RSCb)
        LMTB = pool.tile([128, NBLK, 128], BF16)
        nc.sync.dma_start(out=LMTB[:], in_=lmt_d[:])
        SCIDX = pool.tile([128, NBLK, 2, 128], I16)
        nc.sync.dma_start(out=SCIDX[:], in_=scidx_d[:])

        if STAGE >= 1:
            ll_mlp = nc.gpsimd.load_library(library_config.mlp)
        gathers = []

        PMG = pool.tile([128, 256], F32)                 # [b, i]; /32 folded into wbc
        if STAGE < 1:
            nc.vector.memset(PMG[:], 0.001)

        def emit_pair_gathers(hf):
            GP = pool.tile([128, 128, 64], F32, tag=("BIG" if hf == 0 else "GP1"),
                           name=f"GP{hf}")
            for cc in range(4):
                c = hf * 4 + cc
                for q in range(4):
                    g = nc.gpsimd.dma_gather(
                        out_ap=GP[:, cc * 32 + q * 8:cc * 32 + (q + 1) * 8, :],
                        in_ap=pairs_d[c * 32768:(c + 1) * 32768, :],
                        idxs_ap=PGIDX[:, c, q, :],
                        num_idxs=1024,
                        num_idxs_reg=1024,
                        elem_size=64,
                    )
                    add_dep_helper(g.ins, ll_mlp.ins, reason="gather needs mlp lib")
                    gathers.append(g)
            return GP

        def emit_pair_reduce(GP, hf, after=None):
            R2 = psmall.tile([128, 128, 2], F32, tag="R2", name=f"R2{hf}")
            for q in range(4):
                sl_q = slice(q * 32, (q + 1) * 32)
                r = nc.vector.reduce_sum(
                    R2[:, sl_q, :], GP[:, sl_q, :].rearrange("p i (t f) -> p i t f", t=2),
                    axis=AX.X)
                if after is not None:
                    add_dep_helper(r.ins, after.ins, reason="hold big reduce off the DVE greedy slot")
            sl = slice(hf * 128, (hf + 1) * 128)
            nc.vector.tensor_tensor(out=PMG[:, sl], in0=R2[:, :, 1], in1=R2[:, :, 0], op=ALU.subtract)
            nc.vector.tensor_tensor(out=PMG[:, sl], in0=PMG[:, sl], in1=PGPAR[:, sl], op=ALU.mult)
            nc.vector.tensor_tensor(out=PMG[:, sl], in0=PMG[:, sl], in1=R2[:, :, 0], op=ALU.add)

        # gathers for half 0 start immediately (Pool engine only)
        GP0 = emit_pair_gathers(0) if STAGE >= 1 else None

        # ---------------- LN1 (emitted before pair reduces: DVE order) -----
        def layernorm(dst, src, nt, nm, toff=0, tail=False):
            USE_BN = os.environ.get("KBN", "1") == "1"
            if USE_BN:
                BNS = psmall.tile([128, nt, 6], F32, tag="BNS", name=f"BNS{nm}")
                MV = psmall.tile([128, nt, 2], F32, tag="MV", name=f"MV{nm}")
                for t in range(nt):
                    nc.vector.bn_stats(BNS[:, t, :], src[:, t, :])
                    nc.vector.bn_aggr(MV[:, t, :], BNS[:, t, :])
                MU = MV[:, :, 0]
                VAR = psmall.tile([128, nt], F32, tag="VAR", name=f"VAR{nm}")
                nc.vector.tensor_scalar_add(out=VAR[:], in0=MV[:, :, 1], scalar1=1e-5)
            else:
                SQ = pool.tile([128, nt, C], dst.tensor.dtype, tag="SB1", name=f"SQ{nm}")
                nc.scalar.activation(SQ[:], src[:], AF.Square)
                RS1 = psmall.tile([128, nt], F32, tag="RS1", name=f"RS1{nm}")
                RS2 = psmall.tile([128, nt], F32, tag="RS2", name=f"RS2{nm}")
                nc.vector.reduce_sum(RS1[:], src[:], axis=AX.X)
                nc.vector.reduce_sum(RS2[:], SQ[:], axis=AX.X)
                MUt = psmall.tile([128, nt], F32, tag="MU", name=f"MU{nm}")
                nc.vector.tensor_scalar_mul(out=MUt[:], in0=RS1[:], scalar1=1.0 / C)
                MU = MUt[:]
                VAR = psmall.tile([128, nt], F32, tag="VAR", name=f"VAR{nm}")
                nc.vector.tensor_scalar_mul(out=VAR[:], in0=RS2[:], scalar1=1.0 / C)
                MSQ = psmall.tile([128, nt], F32, tag="MSQ", name=f"MSQ{nm}")
                nc.vector.tensor_tensor(out=MSQ[:], in0=MUt[:], in1=MUt[:], op=ALU.mult)
                nc.vector.tensor_tensor(out=VAR[:], in0=VAR[:], in1=MSQ[:], op=ALU.subtract)
                nc.vector.tensor_scalar_add(out=VAR[:], in0=VAR[:], scalar1=1e-5)
            RSTD = psmall.tile([128, nt], F32, tag="RSTD", name=f"RSTD{nm}")
            if not tail:
                nc.vector.reciprocal(RSTD[:], VAR[:])
                nc.scalar.activation(RSTD[:], RSTD[:], AF.Sqrt)
            else:
                # tail: avoid an act-table switch (Exp/Sigmoid live there) —
                # fast inverse sqrt + two Newton steps on DVE ([128, nt] tiny)
                NT1 = psmall.tile([128, nt], F32, tag="NT1", name=f"NT1{nm}")
                NT2 = psmall.tile([128, nt], F32, tag="NT2", name=f"NT2{nm}")
                I32 = mybir.dt.int32
                nc.vector.tensor_scalar(
                    out=RSTD[:].bitcast(I32), in0=VAR[:].bitcast(I32),
                    scalar1=1, scalar2=None, op0=ALU.logical_shift_right)
                nc.vector.tensor_scalar(
                    out=RSTD[:].bitcast(I32), in0=RSTD[:].bitcast(I32),
                    scalar1=-1, scalar2=0x5F3759DF, op0=ALU.mult, op1=ALU.add)
                for _ in range(2):
                    nc.vector.tensor_tensor(out=NT1[:], in0=VAR[:], in1=RSTD[:], op=ALU.mult)
                    nc.vector.tensor_tensor(out=NT1[:], in0=NT1[:], in1=RSTD[:], op=ALU.mult)
                    nc.vector.tensor_scalar(
                        out=NT2[:], in0=NT1[:], scalar1=-0.5, scalar2=1.5,
                        op0=ALU.mult, op1=ALU.add)
                    nc.vector.tensor_tensor(out=RSTD[:], in0=RSTD[:], in1=NT2[:], op=ALU.mult)
            MB = psmall.tile([128, nt], F32, tag="MB", name=f"MB{nm}")
            nc.vector.tensor_tensor(out=MB[:], in0=MU, in1=RSTD[:], op=ALU.mult)
            last = None
            for t in range(nt):
                last = nc.vector.tensor_scalar(
                    out=dst[:, t + toff, :] if toff else dst[:, t, :], in0=src[:, t, :],
                    scalar1=RSTD[:, t:t + 1], scalar2=MB[:, t:t + 1],
                    op0=ALU.mult, op1=ALU.subtract)
            return last

        XLN = pool.tile([128, 16, C], BF16, tag="GSG", name="XLN")
        ln_a = layernorm(XLN, X, 16, "a")
        XLNO = pool.tile([128, NBLK, C], BF16)
        ln_b = layernorm(XLNO, XOWN, NBLK, "b")

        # pair reduce half 0 (held until LN1 applies are done so the greedy
        # DVE scheduler cannot slot the 8.6us reduce into an LN dep gap)
        if STAGE >= 1:
            emit_pair_reduce(GP0, 0, after=ln_b)
            GP1 = emit_pair_gathers(1)

        # ---------------- transposes ----------------
        XT = pool.tile([128, 2, N], BF16, tag="XT")
        for t in range(16):
            for u in range(2):
                tp = psT.tile([128, 128], BF16, tag="tp", name="tp")
                nc.tensor.transpose(out=tp[:], in_=XLN[:, t, u * 128:(u + 1) * 128], identity=identb[:])
                evac(XT[:, u, t * 128:(t + 1) * 128], tp[:], 1)
        XQT = pool.tile([128, 2, NSH], BF16)
        for t in range(NBLK):
            for u in range(2):
                tp = psT.tile([128, 128], BF16, tag="tp", name="tp")
                nc.tensor.transpose(out=tp[:], in_=XLNO[:, t, u * 128:(u + 1) * 128], identity=identb[:])
                evac(XQT[:, u, t * 128:(t + 1) * 128], tp[:], 1)

        # ---------------- K^T, Q^T (V deferred until after scores) --------
        KT = pool.tile([128, 2, N], BF16, tag="KT")
        QT = pool.tile([128, 2, NSH], BF16)
        for ch in range(2):
            for jg in range(2):
                kp = psM.tile([128, 1024], F32, tag="mm", name="kp")
                for jh in range(2):
                    for u in range(2):
                        nc.tensor.matmul(
                            kp[:, jh * 512:(jh + 1) * 512], WK[:, u, ch * 128:(ch + 1) * 128],
                            XT[:, u, (jg * 2 + jh) * 512:(jg * 2 + jh + 1) * 512],
                            start=(u == 0), stop=(u == 1))
                evac(KT[:, ch, jg * 1024:(jg + 1) * 1024], kp[:], jg)
            qp = psM.tile([128, 1024], F32, tag="mm", name="qp")
            for u in range(2):
                nc.tensor.matmul(
                    qp[:, :NSH], WQ[:, u, ch * 128:(ch + 1) * 128],
                    XQT[:, u, :],
                    start=(u == 0), stop=(u == 1))
            evac(QT[:, ch, :], qp[:, :NSH], ch)

        # ---------------- scores blk0, then pair tail, then scores blk1 ---
        PMGT = pool.tile([128, NBLK, 128], BF16)
        BIAS = pool.tile([128, NBLK, 8, 128], BF16)

        def emit_pair_tail(after=None):
            if STAGE >= 1:
                emit_pair_reduce(GP1, 1, after=after)
            for blk in range(NBLK):
                tp = psT.tile([128, 128], F32, tag="tp", name="tp")
                nc.tensor.transpose(out=tp[:], in_=PMG[:, blk * 128:(blk + 1) * 128], identity=identf[:])
                evac(PMGT[:, blk, :], tp[:], 1)
            for blk in range(NBLK):
                for h in range(H):
                    nc.vector.tensor_scalar_mul(
                        out=BIAS[:, blk, h, :], in0=PMGT[:, blk, :], scalar1=WBC[:, h:h + 1])
                    nc.vector.tensor_tensor(
                        out=BIAS[:, blk, h, :], in0=BIAS[:, blk, h, :], in1=LMTB[:, blk, :],
                        op=ALU.add)

        def emit_scores(blk):
            S_sb = pool.tile([128, 8, 2048], BF16, tag=("BIG" if blk == 0 else "SB1"),
                             name=f"S_sb{blk}")
            last = None
            for h in range(H):
                qs = QT[(h % 4) * 32:(h % 4) * 32 + 32, h // 4, blk * 128:(blk + 1) * 128]
                for jg in range(2):
                    sp = psM.tile([128, 1024], F32, tag="mm", name="sp")
                    for jh in range(2):
                        nc.tensor.matmul(
                            sp[:, jh * 512:(jh + 1) * 512], qs,
                            KT[(h % 4) * 32:(h % 4) * 32 + 32, h // 4,
                               (jg * 2 + jh) * 512:(jg * 2 + jh + 1) * 512],
                            start=True, stop=True, tile_position=((h % 4) * 32, 0))
                    e = evac(S_sb[:, h, jg * 1024:(jg + 1) * 1024], sp[:], h + jg)
                    if (h + jg) % 2 == 0:
                        last = e
            return S_sb, last

        S_sbs = []
        s0_last = None
        if STAGE >= 2:
            s0, s0_last = emit_scores(0)
            S_sbs.append(s0)
        emit_pair_tail(after=s0_last)
        if STAGE >= 2:
            s1, _ = emit_scores(1)
            S_sbs.append(s1)

        # ---------------- V^T + VR (needed only from the AV phase) --------
        VTB = pool.tile([128, 2, N], BF16, tag="VTB")
        for ch in range(2):
            for jg in range(2):
                vp = psM.tile([128, 1024], F32, tag="mm", name="vp")
                for jh in range(2):
                    for u in range(2):
                        nc.tensor.matmul(
                            vp[:, jh * 512:(jh + 1) * 512], WV[:, u, ch * 128:(ch + 1) * 128],
                            XT[:, u, (jg * 2 + jh) * 512:(jg * 2 + jh + 1) * 512],
                            start=(u == 0), stop=(u == 1))
                evac(VTB[:, ch, jg * 1024:(jg + 1) * 1024], vp[:], jg + 1)
        VR = pool.tile([128, 16, C], BF16, tag="VR")
        for ch in range(2):
            nc.sync.dma_start(out=VR[:, :, ch * 128:(ch + 1) * 128], in_=VTB[:, ch, :], transpose=True)

        # W1/W2/B1E loads early, while the SP queue is idle
        B1E = pool.tile([128, 1024], BF16, tag="PGIDX", name="B1E")
        nc.sync.dma_start(out=B1E[:], in_=b1e_d[:])
        W1 = pool.tile([128, 2, 4 * C], BF16, tag="GSG", name="W1")
        nc.sync.dma_start(out=W1[:], in_=w1t_d[:].rearrange("(u p) c -> p u c", p=128))
        W2 = pool.tile([128, 8, C], BF16, tag="X", name="W2")
        nc.sync.dma_start(out=W2[:], in_=w2t_d[:].rearrange("(u p) c -> p u c", p=128))

        chain_depth = int(os.environ.get("KCHAIN", "2"))
        if chain_depth > 0:
            for n in range(chain_depth, len(gathers)):
                add_dep_helper(gathers[n].ins, gathers[n - chain_depth].ins, sync=True,
                               reason="swdge ring reclaim chain")

        # gpsimd switches libraries after ALL gathers
        if STAGE >= 2:
            ll_ls = nc.gpsimd.load_library(library_config.local_scatter)
            for g in gathers:
                add_dep_helper(ll_ls.ins, g.ins, reason="lib switch after gathers")

        # ------------- reverse-scatter select (both blocks back-to-back) ---
        STTs = []
        for blk in range(NBLK if STAGE >= 2 else 0):
            S_sb = S_sbs[blk]
            STT = pool.tile([128, 2, 8, 128], BF16, tag=f"STT{blk}", name=f"STT{blk}")
            for h in range(H):
                for half in range(2):
                    ls = nc.gpsimd.local_scatter(
                        out_ap=STT[:, half, h, :],
                        data_ap=S_sb[:, h, half * 1024:(half + 1) * 1024],
                        idxs_ap=RSCs[blk][:, half, :],
                        channels=128,
                        num_elems=128,
                        num_idxs=1024,
                    )
                    add_dep_helper(ls.ins, ll_ls.ins, reason="rev scatter needs ls lib")
            STTs.append(STT)

        # ------------- softmax for both blocks (free b axis, [i, h, b]) ----
        ATT = pool.tile([128, 2, NSH], BF16, tag="ATT")
        if STAGE < 3:
            nc.vector.memset(ATT[:], 0.001)
        WTs = []
        for blk in range(NBLK if STAGE >= 2 else 0):
            STT = STTs[blk]
            nc.vector.tensor_tensor(
                out=STT[:, 0], in0=STT[:, 0], in1=BIAS[:, blk], op=ALU.add)
            nc.vector.tensor_tensor(
                out=STT[:, 0], in0=STT[:, 0], in1=STT[:, 1], op=ALU.add)
            E = pool.tile([128, 8, 128], BF16, tag="VTB", name=f"E{blk}")
            nc.scalar.activation(E[:], STT[:, 0], AF.Exp)
            D = psmall.tile([128, 8], F32, tag="D", name=f"D{blk}")
            nc.vector.reduce_sum(D[:], E[:], axis=AX.X)
            DR = psmall.tile([128, 8], F32, tag="DR", name=f"DR{blk}")
            nc.vector.reciprocal(DR[:], D[:])
            WT = pool.tile([128, 8, 128], BF16, tag=f"WT{blk}", name=f"WT{blk}")
            for h in range(H):
                nc.vector.tensor_scalar_mul(
                    out=WT[:, h, :], in0=E[:, h, :], scalar1=DR[:, h:h + 1])
            WTs.append(WT)

        # ------------- per block: scatter + AV + tail ----------------------
        for blk in range(NBLK if STAGE >= 3 else 0):
            WT = WTs[blk]
            # scatter to dense P and AV
            PD = pool.tile([128, 8, 2048], BF16, tag=("BIG" if blk == 0 else "SB1"),
                           name=f"PD{blk}")
            for h in range(H):
                for half in range(2):
                    ls = nc.gpsimd.local_scatter(
                        out_ap=PD[:, h, half * 1024:(half + 1) * 1024],
                        data_ap=WT[:, h, :],
                        idxs_ap=SCIDX[:, blk, half, :],
                        channels=128,
                        num_elems=1024,
                        num_idxs=128,
                    )
                    add_dep_helper(ls.ins, ll_ls.ins, reason="scatter needs ls lib")

            for hg in range(2):
                for k in range(4):
                    h = hg * 4 + k
                    PT = pool3.tile([128, 16, 128], BF16, tag="MS", name="PT")
                    dma_eng = (nc.sync, nc.scalar)[h % 2]
                    dma_eng.dma_start(out=PT[:], in_=PD[:, h, :], transpose=True)
                    av = psA.tile([128, 128], F32, tag="av", name="av")
                    for jh in range(16):
                        nc.tensor.matmul(
                            av[:], VR[:, jh, hg * 128:(hg + 1) * 128],
                            PT[:, jh, :],
                            start=(jh == 0), stop=(jh == 15))
                    evac(ATT[k * 32:(k + 1) * 32, hg, blk * 128:(blk + 1) * 128],
                         av[k * 32:(k + 1) * 32, :], k)

            # ---- per-block tail: Wout + residual (row space) + LN2 + MLP --
            bs = slice(blk * 128, (blk + 1) * 128)
            Y1 = pool.tile([128, 1, C], F32, tag=f"Y1_{blk}", name=f"Y1_{blk}")
            YS = pool.tile([128, 2, 128], F32, tag=f"Y1T{blk}", name=f"YS{blk}")
            for ch in range(2):
                op_ = psM.tile([128, 1024], F32, tag="mm", name="op")
                for u in range(2):
                    nc.tensor.matmul(
                        op_[:, :128], WO[:, u, ch * 128:(ch + 1) * 128],
                        ATT[:, u, bs],
                        start=(u == 0), stop=(u == 1))
                evac(YS[:, ch, :], op_[:, :128], ch)
                tp = psT.tile([128, 128], F32, tag="tp", name="tp")
                nc.tensor.transpose(out=tp[:], in_=YS[:, ch, :], identity=identf[:])
                nc.vector.tensor_tensor(
                    out=Y1[:, 0, ch * 128:(ch + 1) * 128], in0=tp[:],
                    in1=XOWN[:, blk, ch * 128:(ch + 1) * 128], op=ALU.add)
            H2 = pool.tile([128, 1, C], F32, tag="H2", name=f"H2_{blk}")
            layernorm(H2, Y1, 1, f"c{blk}", tail=True)
            H2T = pool.tile([128, 2, 128], BF16, tag=f"H2T{blk}", name=f"H2T{blk}")
            for ch in range(2):
                tp = psT.tile([128, 128], F32, tag="tp", name="tp")
                nc.tensor.transpose(out=tp[:], in_=H2[:, 0, ch * 128:(ch + 1) * 128], identity=identf[:])
                evac(H2T[:, ch, :], tp[:], ch)

            M1 = pool.tile([128, 8, 128], BF16, tag=f"M1_{blk}", name=f"M1_{blk}")
            TM = pool.tile([128, 1024], F32, tag="KT", name=f"TM{blk}")
            SG = pool.tile([128, 1024], BF16, tag="STT0", name=f"SG{blk}")
            for half in range(2):
                mp = psM.tile([128, 1024], F32, tag="mm", name="mp")
                hs = slice(half * 512, (half + 1) * 512)
                for mq in range(4):
                    mc = half * 4 + mq
                    for u in range(2):
                        nc.tensor.matmul(
                            mp[:, mq * 128:(mq + 1) * 128], W1[:, u, mc * 128:(mc + 1) * 128],
                            H2T[:, u, :],
                            start=(u == 0), stop=(u == 1))
                nc.vector.tensor_tensor(out=TM[:, hs], in0=mp[:, :512], in1=B1E[:, hs], op=ALU.add)
                nc.scalar.activation(SG[:, hs], TM[:, hs], AF.Sigmoid)
                nc.vector.tensor_tensor(
                    out=M1[:].rearrange("p a b -> p (a b)")[:, hs], in0=SG[:, hs],
                    in1=TM[:, hs], op=ALU.mult)
            OUT = pool.tile([128, 1, C], F32, tag="OUT", name=f"OUT{blk}")
            YB = pool.tile([128, 2, 128], F32, tag=f"YT{blk}", name=f"YB{blk}")
            for ch in range(2):
                yp = psM.tile([128, 1024], F32, tag="mm", name="yp")
                for mc in range(8):
                    nc.tensor.matmul(
                        yp[:, :128], W2[:, mc, ch * 128:(ch + 1) * 128],
                        M1[:, mc, :],
                        start=(mc == 0), stop=(mc == 7))
                nc.vector.tensor_scalar_add(out=YB[:, ch, :], in0=yp[:, :128], scalar1=B2[:, ch:ch + 1])
                tp = psT.tile([128, 128], F32, tag="tp", name="tp")
                nc.tensor.transpose(out=tp[:], in_=YB[:, ch, :], identity=identf[:])
                nc.vector.tensor_tensor(
                    out=OUT[:, 0, ch * 128:(ch + 1) * 128], in0=tp[:],
                    in1=Y1[:, 0, ch * 128:(ch + 1) * 128], op=ALU.add)
            nc.sync.dma_start(
                out=out_d[blk * 128:(blk + 1) * 128], in_=OUT[:, 0, :])

    nc.compile()
    return nc


# ======================= host side =======================

def _wrap16(flat):
    """dma_gather index layout: idx k at [k%16, k//16], replicated x8 groups."""
    n = flat.shape[0]
    arr = flat.reshape(n // 16, 16).T.astype(np.int16)
    return np.tile(arr, (8, 1))


def _host_prep(core, x, pair_emb, block_index, Wq, Wk, Wv, Wb, Wout, W1, b1, W2, b2):
    i0 = core * NSH
    idx = np.asarray(block_index[i0:i0 + NSH]).astype(np.int64)   # [256, 128]

    pgidx = np.zeros((128, 8, 4, 64), np.int16)
    for c in range(8):
        sub = idx[c * 32:(c + 1) * 32]                            # [32, 128]
        flat = (np.arange(32)[:, None] * 1024 + sub // 2).reshape(-1)
        for q in range(4):
            pgidx[:, c, q, :] = _wrap16(flat[q * 1024:(q + 1) * 1024])
    pgpar = np.ascontiguousarray((idx % 2).astype(np.float32).T)  # [b, i]

    rsc = np.full((128, NBLK, 2, 1024), -1, np.int16)
    lmt = np.zeros((128, NBLK, 128), ml_dtypes.bfloat16)
    scidx = np.zeros((128, NBLK, 2, 128), np.int16)
    for blk in range(NBLK):
        sub = idx[blk * 128:(blk + 1) * 128]                      # [128 i, 128 b]
        for i in range(128):
            row = sub[i]
            uniq, first_pos, counts = np.unique(row, return_index=True, return_counts=True)
            lmrow = np.full(128, -30.0, np.float32)
            scrow = np.full(128, -1, np.int64)
            lmrow[first_pos] = np.log(counts.astype(np.float64)).astype(np.float32)
            scrow[first_pos] = uniq
            lmt[i, blk, :] = lmrow.astype(ml_dtypes.bfloat16)
            rsc[i, blk, uniq // 1024, uniq % 1024] = first_pos.astype(np.int16)
            for half in range(2):
                sc = np.where((scrow >= 1024 * half) & (scrow < 1024 * (half + 1)),
                              scrow - 1024 * half, -1)
                scidx[i, blk, half, :] = sc.astype(np.int16)

    scale = 1.0 / math.sqrt(DH)
    fp = np.float32
    bf = ml_dtypes.bfloat16
    feeds = {
        "x": np.ascontiguousarray(np.asarray(x, fp)).astype(bf),
        "xown": np.ascontiguousarray(np.asarray(x, fp)[i0:i0 + NSH]),
        "pairs": np.ascontiguousarray(
            np.asarray(pair_emb[i0:i0 + NSH], fp).reshape(NSH * 1024, 64)),
        "pgidx": pgidx, "pgpar": pgpar, "rsc": rsc, "lmt": lmt, "scidx": scidx,
        "wqt": np.ascontiguousarray(np.asarray(Wq, fp).T * scale).astype(bf),
        "wkt": np.ascontiguousarray(np.asarray(Wk, fp).T).astype(bf),
        "wvt": np.ascontiguousarray(np.asarray(Wv, fp).T).astype(bf),
        "wot": np.ascontiguousarray(np.asarray(Wout, fp).T).astype(bf),
        "w1t": np.ascontiguousarray(np.asarray(W1, fp).T).astype(bf),
        "w2t": np.ascontiguousarray(np.asarray(W2, fp).T).astype(bf),
        "b1e": np.tile(np.asarray(b1, fp).reshape(8, 128).T.reshape(128, 8, 1),
                       (1, 1, 128)).reshape(128, 1024).astype(ml_dtypes.bfloat16),
        "b2p": np.ascontiguousarray(np.asarray(b2, fp).reshape(2, 128).T),
        "wbc": np.tile(np.asarray(Wb, fp).reshape(1, 8) / CP, (128, 1)),
    }
    return feeds


_NC = None


def kernel(**inputs):
    global _NC
    from concourse.bass_utils import run_bass_kernel_spmd
    if _NC is None:
        _NC = build_nc()
    in_maps = [_host_prep(core, **inputs) for core in range(NCORES)]
    r = run_bass_kernel_spmd(_NC, in_maps, core_ids=list(range(NCORES)))
    out = np.concatenate([np.asarray(r.results[i]["out"]).reshape(NSH, C)
                          for i in range(NCORES)], axis=0)
    return out.astype(np.float32)
